# revision 37
# baseline (speedup 1.0000x reference)
"""DTSemNet forward (nn_DTSemNet_54528904790526) on 8 TRN2 NeuronCores.

Math: the reference computes
    x = in_x @ W1.T + b1                       [B, 2047]
    h = [relu(x), relu(-x)]                    [B, 4094]
    z = h @ L.T                                [B, 2048]   (frozen 0/1 leaf routing)
    out[b, a] = max over leaves ell with (ell % 10 == a) of z[b, ell]

L is the complete-binary-tree path matrix, so
    z[b, ell] = sum_i |x_i|  -  sum_{path nodes} penalty,
    penalty   = relu(-x_node) going left, relu(x_node) going right,
replacing the dense [B,4094]x[4094,2048] matmul with an 11-level tree DP
on the vector engine (see kernel_fp32r_baseline.py for the derivation).

This version runs the linear1 matmul in fp8 (e4m3) with
MatmulPerfMode.DoubleRowSwInterleave: 2 fp8 weights per PE cell ->
256-deep contraction per instruction; the software-interleaved
(pair-interleaved, column-reversed) stationary layout loads weights
contiguously. Inputs are quantized host-side: activations cast straight
to e4m3 (|x|<6 << 240, TRN e4m3 infs at 256), weights and bias
pre-scaled by 2^7 (exact) to clear the e4m3 subnormal floor at 2^-6;
the relu undoes the scale exactly via the activation scale (+-2^-7).
Measured end-to-end error: max rel 8.1e-3 vs the 2e-2 gate (matmul
accumulates in fp32 PSUM; e4m3 products are exact in fp32).

Per-core shard: batch rows (data parallel over 8 cores, 2048 rows each).
Per 128-row batch tile: 8 SwInterleave k-steps x 4 chunk matmuls
(k-outer so 4 consecutive matmuls share the stationary) accumulate
x*2^7 into one [128,2048] PSUM tile (4 banks); the bias row is folded
in as a K=1 ones-row NORMAL-mode (FWL) matmul per chunk at 4 PE
row-group positions -- HW-measured ~13 us/sweep faster than the
DoubleRow bias matmuls of the earlier version. Two full-width
activations produce pen = relu(+-x) bf16 with accumulated row-sums
(S_abs). The 11-level tree DP runs on DVE in bf16 at ~0.86 elem/cyc
(the rotate layout that keeps tree-order m within halves forces a
non-contiguous innermost par operand, which blocks DVE 2X packing;
every contiguous alternative bit-reverses the leaf order and breaks
the mod-5 group reduce -- verified impossible with affine APs). The
group-min is a 3-stage fold chain 1020->510->255->130 (fold offsets
510, 255, 130 are all = 0 mod 5, preserving residues) + leftover
fixups + one 4-D strided reduce [s][r=5][j=26] + the final
(-min+sacc0)+sacc1 combine, emitted inline (tail_depth=0: with the
folds this small, deferring the tail one tile measured slightly worse
-- longer dp-tile lifetimes -- where it had been a win when the reduce
read 1020 elements). Folds 2+3 measured ~8-11 us/sweep combined.

Corrected HW model (this session, measured by deltas + NEFF disasm +
CoreSim): LDWEIGHTS is NOT the matmul floor -- eliding 24/32 redundant
LDWs per tile (elide_redundant_ldweights below, kept on) changed
nothing; the stream is bound by the DoubleRow MATMUL ALU time itself,
~175 ns marginal per 512-col MM (k-slope probes). Chunk-outer MM order
(per-MM stationary reload) costs +26 ns/MM => k-outer stays. Pool (Q7)
is useless for compute here: TT add 0.5 elem/cyc, no TT-min/max, no
PSUM reads, TS ~0.07 elem/cyc -- it only carries the output DMAs.
Offloading DP levels/halves to Pool measured strictly worse.

Measured on this container (For_i repeat-loop deltas, fresh process
per measurement; device has +-5% state noise): ~141-148 us/core-sweep
vs ~150 us for the previous version and 248.6 us for the fp32r
baseline. mmpure floor ~112 us (main MMs ~91 us + fixed ~21 us of
DMA/drain/gaps); the For_i all-engine barrier per iteration makes each
sweep pay the full ACT+DVE drain (~12 us), which a single harness
invocation pays once too. Other dead ends measured this session:
deeper pen/dp/sm/xt rings (no gain; DVE-saturated per CoreSim -- ACT
idle is not binding), xt DMA on an alternate (ACT) queue (+19 us),
half-width ACT splits (+3 us), and pre-materializing the DP par
duplication via SBUF->SBUF strided DMA to unlock DVE 2X packing
(the 2-byte-stride DMA runs at ~100 ns/element -- useless).
"""
import sys

sys.path.insert(0, "/opt/trn_rl_repo")
from contextlib import ExitStack

import numpy as np
import ml_dtypes

import concourse.bass as bass
import concourse.tile as tile
from concourse import bacc, mybir
from concourse.bass_utils import run_bass_kernel_spmd

# problem shape (hardcoded per contract)
B = 16384
D = 2048
N = 2047          # internal nodes
NP = 2048         # N padded (zero column 2047)
HEIGHT = 11
NL = 2048         # leaves
OUT = 10
NCORES = 8
BC = B // NCORES  # batch rows per core (2048)
KT = D // 128     # 16 k-subtiles of 128 contraction rows
KT2 = KT // 2     # 8 DoubleRow k-steps of 256
BT = BC // 128    # 16 batch tiles per core
CHUNKS = [(0, 512), (512, 1024), (1024, 1536), (1536, 2048)]
WSCALE = 128.0    # weight pre-scale 2^7 (exact in fp8/fp32)

F8 = ml_dtypes.float8_e4m3   # TRN fp8e4: inf at S.1111.000, max normal 240

f32 = mybir.dt.float32
fp8 = mybir.dt.float8e4
bf16 = mybir.dt.bfloat16
ADD = mybir.AluOpType.add
MIN = mybir.AluOpType.min
SUB = mybir.AluOpType.subtract
MULT = mybir.AluOpType.mult
RELU = mybir.ActivationFunctionType.Relu
AXX = mybir.AxisListType.X
DR = mybir.MatmulPerfMode.DoubleRow
SW = mybir.MatmulPerfMode.DoubleRowSwInterleave


def elide_redundant_ldweights(nc):
    """Drop InstLdweights that reload the stationary already in the PE.

    The Tile scheduler emits one Ldweights+Matmult pair per matmul; the 4
    chunk matmuls of one k-step share the same stationary (x3[:, k]), so
    3 of every 4 Ldweights reload identical weights (~107ns each on the
    serial PE queue). Walrus lowers InstMatmult to a bare MATMUL that uses
    whatever stationary is loaded, so removing a redundant Ldweights is
    safe when (a) its weights AP + tile geometry match the previously
    loaded one with only Matmults in between on the PE queue, and (b) it
    carries no semaphore waits/updates.
    """
    PE = mybir.EngineType.PE
    removed = 0
    for blk in nc.main_func.blocks:
        il = blk.instructions
        cur_sig = None
        drop = set()
        for i in il:
            if getattr(i, "engine", None) != PE:
                continue
            t = type(i).__name__
            if t == "InstLdweights":
                sig = (str(i.ins[0]), str(i.tile_position),
                       str(i.tile_size), str(i.perf_mode),
                       str(i.is_transpose))
                si = i.sync_info
                clean = si is None or (len(si.on_wait) == 0
                                       and len(si.on_update) == 0)
                if clean and sig == cur_sig:
                    drop.add(id(i))
                    removed += 1
                else:
                    cur_sig = sig
            elif t == "InstMatmult":
                pass          # uses loaded stationary, doesn't change it
            else:
                cur_sig = None  # unknown PE instruction: invalidate
        if drop:
            blk.instructions = [i for i in il if id(i) not in drop]
    return removed


def build_kernel(bt=BT, reps=1, loop_reps=None, mode="full", elide_ldw=True,
                 kt2=KT2, korder="k", biasfirst=False,
                 pool_split=False, defer_fold=False, fold2x=True,
                 bias_normal=True, tail_depth=0, pool_dmax=0, fold3x=True,
                 act_split=False, xt_bufs=6, pen_bufs=2, dp_bufs=3,
                 sm_bufs=4, xt_alt_queue=False, pair_dp=False):
    """bt: number of batch tiles (128 rows each) this kernel processes.
    reps: python-unrolled repeats of the whole per-tile pipeline.
    loop_reps: device-side For_i repeats (for timing probes).
    mode: "full" | "nodp" (skip tree DP/mins) | "mmonly" (matmuls only)
          | "dponly" (memset pen, DP chain only)."""
    nc = bacc.Bacc("TRN2")
    # in_x fp8 shard, pre-blocked host-side as [bt][128 p][KT j][128 m]
    # with p = contraction row % 128, j = contraction row // 128, so each
    # SBUF partition reads one contiguous 2KB run per batch tile.
    xt = nc.dram_tensor("xt", [bt * 128, KT * 128], fp8, kind="ExternalInput")
    # W1.T * 2^7 (zero col at node 2047), blocked [128 p][KT j][NP n]
    wt = nc.dram_tensor("wt", [128, KT * NP], fp8, kind="ExternalInput")
    # ones rows for the bias matmul: [1.0]*128 | [0.0]*128 per row
    ones = nc.dram_tensor("ones", [4, 2 * 128], fp8, kind="ExternalInput")
    # bias rows: b1 * 2^7 (2048, zero-padded) | zeros(2048) per row
    wtb = nc.dram_tensor("wtb", [4, 2 * NP], fp8, kind="ExternalInput")
    out = nc.dram_tensor("out", [bt * 128, OUT], f32, kind="ExternalOutput")

    with tile.TileContext(nc) as tc, ExitStack() as ctx:
        wt_pool = ctx.enter_context(tc.tile_pool(name="wt", bufs=1))
        xt_pool = ctx.enter_context(tc.tile_pool(name="xt", bufs=xt_bufs))
        ps_pool = ctx.enter_context(tc.tile_pool(name="ps", bufs=2, space="PSUM"))
        pen_pool = ctx.enter_context(tc.tile_pool(name="pen", bufs=pen_bufs))
        dp_pool = ctx.enter_context(tc.tile_pool(name="dp", bufs=dp_bufs))
        sm_pool = ctx.enter_context(tc.tile_pool(name="sm", bufs=sm_bufs))

        # resident weights: [128, KT, NP] fp8 (32KB/partition)
        wts = wt_pool.tile([128, KT * NP], fp8, tag="wt")
        nc.sync.dma_start(wts[:], wt[:, :])
        wt3 = wts[:].rearrange("p (j n) -> p j n", n=NP)
        # bias + ones rows replicated at partitions 0/32/64/96 so the four
        # K=1 bias matmuls can run as concurrent PE row-group tiles
        ones_t = wt_pool.tile([128, 2 * 128], fp8, tag="ones")
        wtb_t = wt_pool.tile([128, 2 * NP], fp8, tag="wtb")
        for rg in range(4):
            nc.sync.dma_start(ones_t[32 * rg:32 * rg + 1, :], ones[rg:rg + 1, :])
            nc.sync.dma_start(wtb_t[32 * rg:32 * rg + 1, :], wtb[rg:rg + 1, :])
        ones3 = ones_t[:].rearrange("p (i m) -> p i m", m=128)
        wtb3 = wtb_t[:].rearrange("p (i n) -> p i n", n=NP)

        def body():
            pending = []
            for t in range(bt):
                c_lo = t * 128
                c_hi = (t + 1) * 128
                xt_t = xt_pool.tile([128, KT * 128], fp8, tag="xt")
                if xt_alt_queue and t % 2 == 1:
                    nc.scalar.dma_start(xt_t[:], xt[c_lo:c_hi, :])
                else:
                    nc.sync.dma_start(xt_t[:], xt[c_lo:c_hi, :])
                # SwInterleave stationary layout: per k-step, pairs
                # (sub0, sub1) interleaved per batch column, columns reversed
                x3 = xt_t[:].rearrange("p (j t i) -> p j t i", i=2, t=128)

                pen = pen_pool.tile([128, 2 * NP], bf16, tag="pen")
                sacc = sm_pool.tile([128, 4], f32, tag="sacc")

                if mode == "dponly":
                    # fill pen/sacc cheaply so the DP chain is isolated
                    nc.vector.memset(pen[:], 0.5)
                    nc.vector.memset(sacc[:], 1.0)
                else:
                    ps = ps_pool.tile([128, NP], f32, tag="ps")
                    if mode == "mmnodma":
                        nc.vector.memset(xt_t[:], 0.25)
                    skip_bias = mode in ("mmnob", "mmnobias")
                    if biasfirst and not skip_bias:
                        for ci, (c0, c1) in enumerate(CHUNKS):
                            bp = 32 * ci
                            nc.tensor.matmul(
                                ps[:, c0:c1],
                                ones3[bp:bp + 1, :, :],
                                wtb3[bp:bp + 1, :, c0:c1],
                                start=True, stop=False,
                                perf_mode=DR,
                                tile_position=(bp, 0),
                            )
                    if korder == "chunk":
                        # chunk-outer: each chunk's 8 k-steps consecutive
                        for ci, (c0, c1) in enumerate(CHUNKS):
                            for k in range(kt2):
                                nc.tensor.matmul(
                                    ps[:, c0:c1],
                                    x3[:, k],
                                    wt3[:, 2 * k:2 * k + 2, c0:c1],
                                    start=(k == 0 and not biasfirst),
                                    stop=(k == kt2 - 1
                                          and (biasfirst or skip_bias)),
                                    perf_mode=SW,
                                )
                    else:
                        # k-outer order: 4 consecutive matmuls share the
                        # stationary xt block
                        for k in range(kt2):
                            for ci, (c0, c1) in enumerate(CHUNKS):
                                nc.tensor.matmul(
                                    ps[:, c0:c1],
                                    x3[:, k],
                                    wt3[:, 2 * k:2 * k + 2, c0:c1],
                                    start=(k == 0 and not biasfirst),
                                    stop=(k == kt2 - 1
                                          and (biasfirst or skip_bias)),
                                    perf_mode=SW,
                                )
                    if not biasfirst and not skip_bias:
                        for ci, (c0, c1) in enumerate(CHUNKS):
                            bp = 32 * ci
                            if bias_normal:
                                nc.tensor.matmul(
                                    ps[:, c0:c1],
                                    ones3[bp:bp + 1, 0, :],
                                    wtb3[bp:bp + 1, 0, c0:c1],
                                    start=False, stop=True,
                                    tile_position=(bp, 0),
                                )
                            else:
                                nc.tensor.matmul(
                                    ps[:, c0:c1],
                                    ones3[bp:bp + 1, :, :],
                                    wtb3[bp:bp + 1, :, c0:c1],
                                    start=False, stop=True,
                                    perf_mode=DR,
                                    tile_position=(bp, 0),
                                )
                    if mode in ("mmpure", "mmnobias"):
                        continue
                    if mode in ("mmonly", "mmnob", "mmnodma"):
                        outsb = sm_pool.tile([128, OUT], f32, tag="outsb")
                        nc.scalar.copy(outsb[:], ps[:, 0:OUT])
                        nc.sync.dma_start(out[c_lo:c_hi, :], outsb[:])
                        continue
                    # pen = [relu(-x) | relu(x)] with running row-sums;
                    # the 2^-7 scale undoes the weight pre-scale exactly.
                    if act_split:
                        # lo halves first (nodes 0:1024) so the DP levels
                        # 1..9 can start while the hi halves run
                        H2 = NP // 2
                        nc.scalar.activation(
                            pen[:, NP:NP + H2], ps[:, 0:H2], RELU,
                            scale=1.0 / WSCALE, accum_out=sacc[:, 0:1])
                        nc.scalar.activation(
                            pen[:, 0:H2], ps[:, 0:H2], RELU,
                            scale=-1.0 / WSCALE, accum_out=sacc[:, 1:2])
                        nc.scalar.activation(
                            pen[:, NP + H2:2 * NP], ps[:, H2:NP], RELU,
                            scale=1.0 / WSCALE, accum_out=sacc[:, 2:3])
                        nc.scalar.activation(
                            pen[:, H2:NP], ps[:, H2:NP], RELU,
                            scale=-1.0 / WSCALE, accum_out=sacc[:, 3:4])
                        # sacc[0:2] += sacc[2:4] so the final TS still uses
                        # just two scalars
                        nc.vector.tensor_tensor(
                            sacc[:, 0:2], sacc[:, 0:2], sacc[:, 2:4], op=ADD)
                    else:
                        nc.scalar.activation(
                            pen[:, NP:2 * NP], ps[:, 0:NP], RELU,
                            scale=1.0 / WSCALE,
                            accum_out=sacc[:, 0:1],
                        )
                        nc.scalar.activation(
                            pen[:, 0:NP], ps[:, 0:NP], RELU,
                            scale=-1.0 / WSCALE,
                            accum_out=sacc[:, 1:2],
                        )

                if mode in ("nodp", "nomin"):
                    sabs = sm_pool.tile([128, 1], f32, tag="sabs")
                    nc.vector.tensor_tensor(
                        sabs[:, 0:1], sacc[:, 0:1], sacc[:, 1:2], op=ADD)
                if mode == "nodp":
                    outsb = sm_pool.tile([128, OUT], f32, tag="outsb")
                    nc.scalar.copy(outsb[:, 0:1], sabs[:])
                    nc.sync.dma_start(out[c_lo:c_hi, 0:1], outsb[:, 0:1])
                    continue

                # ---- tree DP over 11 levels, split (evens|odds) layout ----
                # One TT per level: out[s, j, u] = par[j, u] + pen[s][n0 + 2j+u]
                # (s = 0 left / 1 right half; parent broadcast via stride-0 dim)
                # level-1 costs are pen[0] (left child) and pen[NP] (right)
                par = pen[:, 0:2 * NP:NP].rearrange("p (j u) -> p j u", u=2)
                pen2 = pen.rearrange("p (s c) -> p s c", s=2)
                lvl = None
                for d in range(1, HEIGHT):
                    w = 1 << d          # number of level-d nodes = parents
                    n0 = w - 1          # first node index of level d
                    nxt = dp_pool.tile([128, 2 * w], bf16, tag=f"lvl{d + 1}")
                    if d == HEIGHT - 1 and pool_split:
                        # final (biggest) level: split halves across engines
                        # -- DVE takes s=0, the otherwise-idle Pool s=1 --
                        # to shed ~1.2us/tile of DVE time. par is shared.
                        for s, eng in ((0, nc.vector), (1, nc.gpsimd)):
                            out3 = nxt[:, s * w:(s + 1) * w].rearrange(
                                "p (j u) -> p j u", u=2)
                            pen3 = pen[:, s * NP + n0:s * NP + n0 + w
                                       ].rearrange("p (j u) -> p j u", u=2)
                            eng.tensor_tensor(out3, par, pen3, op=ADD)
                    else:
                        out4 = nxt[:].rearrange(
                            "p (s j u) -> p s j u", s=2, u=2)
                        pen4 = pen2[:, :, n0:n0 + w].rearrange(
                            "p s (j u) -> p s j u", u=2)
                        par4 = par.rearrange("p (x j) u -> p x j u", x=1)
                        par4 = par4.broadcast_to([128, 2, w // 2, 2])
                        eng = nc.gpsimd if d <= pool_dmax else nc.vector
                        eng.tensor_tensor(out4, par4, pen4, op=ADD)
                    lvl = nxt
                    par = nxt[:].rearrange("p (u j) -> p j u", u=2)

                if mode == "nomin":
                    outsb = sm_pool.tile([128, OUT], f32, tag="outsb")
                    nc.vector.tensor_scalar(
                        outsb[:], lvl[:, 0:OUT], sabs[:, 0:1], -1.0,
                        op0=SUB, op1=MULT,
                    )
                    nc.sync.dma_start(out[c_lo:c_hi, :], outsb[:])
                    continue

                # lvl holds leaf costs [128, 2048]: evens | odds halves.
                # group a=2r+s: min over positions m ≡ r (mod 5) of half s.
                # The ENTIRE min tail (folds + reduce + combine + store) is
                # deferred one tile (software pipelining): the Pool half of
                # the last DP level lands late, and an in-line fold would
                # head-of-line-block the DVE FIFO waiting on it.
                # Fold chain: 1020 -> 510 -> 255 (both 510 and 255 are
                # ≡ 0 mod 5, so residues are preserved), leftover
                # m=1020..1023 (residues 0..3) folded into the matching
                # residue slots, then one 4-D strided reduce
                # [s][r=5][j=51] -> tmp[s*5+r] with negate, and the final
                # (-min + sacc0) + sacc1 combine.
                def do_folds(lvl2, fold2):
                    nc.vector.tensor_tensor(
                        fold2[:, :, 0:510], lvl2[:, :, 0:510],
                        lvl2[:, :, 510:1020], op=MIN,
                    )
                    nc.vector.tensor_tensor(
                        fold2[:, :, 0:4], fold2[:, :, 0:4],
                        lvl2[:, :, 1020:1024], op=MIN,
                    )
                    if fold2x:
                        nc.vector.tensor_tensor(
                            fold2[:, :, 0:255], fold2[:, :, 0:255],
                            fold2[:, :, 255:510], op=MIN,
                        )
                    if fold3x:
                        # 255 -> 130: pairs (i, i+130), 130 ≡ 0 mod 5;
                        # leftover 125:130 (residues 0..4) folds into 0:5
                        nc.vector.tensor_tensor(
                            fold2[:, :, 0:125], fold2[:, :, 0:125],
                            fold2[:, :, 130:255], op=MIN,
                        )
                        nc.vector.tensor_tensor(
                            fold2[:, :, 0:5], fold2[:, :, 0:5],
                            fold2[:, :, 125:130], op=MIN,
                        )

                pre_fold2 = None
                if not defer_fold:
                    lvl2_i = lvl[:].rearrange("p (s c) -> p s c", s=2)
                    fold_i = dp_pool.tile([128, 2 * 512], bf16, tag="fold")
                    pre_fold2 = fold_i[:].rearrange("p (s c) -> p s c", s=2)
                    do_folds(lvl2_i, pre_fold2)

                def min_tail(lvl=lvl, sacc=sacc, c_lo=c_lo, c_hi=c_hi,
                             pre_fold2=pre_fold2):
                    if pre_fold2 is None:
                        lvl2 = lvl[:].rearrange("p (s c) -> p s c", s=2)
                        fold = dp_pool.tile([128, 2 * 512], bf16, tag="fold")
                        fold2 = fold[:].rearrange("p (s c) -> p s c", s=2)
                        do_folds(lvl2, fold2)
                    else:
                        fold2 = pre_fold2
                    nred = 130 if fold3x else (255 if fold2x else 510)
                    tmp = sm_pool.tile([128, 2 * 5], bf16, tag="mins")
                    tmp3 = tmp[:].rearrange("p (s r) -> p s r", s=2)
                    src = fold2[:, :, 0:nred].rearrange(
                        "p s (j r) -> p s r j", r=5)
                    nc.vector.tensor_reduce(
                        tmp3, src, axis=AXX, op=MIN, negate=True)
                    outsb = sm_pool.tile([128, OUT], f32, tag="outsb")
                    # out[:, 2r+s] = (-min[s*5+r] + sacc0) + sacc1
                    out_perm = outsb[:].rearrange("p (r s) -> p s r", s=2)
                    nc.vector.tensor_scalar(
                        out_perm, tmp3, sacc[:, 0:1], sacc[:, 1:2],
                        op0=ADD, op1=ADD,
                    )
                    # out DMA via the idle Pool SWDGE: it depends on the late
                    # DVE tail, and on the SP queue it would head-of-line
                    # block the next tiles' xt input loads
                    nc.gpsimd.dma_start(out[c_lo:c_hi, :], outsb[:])

                pending.append(min_tail)
                if len(pending) > tail_depth:
                    pending.pop(0)()
            for fn in pending:
                fn()

        def body_pair():
            # Pair-batched tail: walrus DVE codegen is TENSOR3D (max 3 free
            # dims), and the level TTs' broadcast par dim cannot merge with
            # a tile dim, so the DP levels stay per-tile. But the fold and
            # reduce ops have no broadcast operands -- their (t, s) dims
            # merge -- so one tail per tile PAIR halves tail op count and
            # semaphore traffic (~6 DVE ops saved per pair).
            for t0 in range(0, bt, 2):
                sacc = sm_pool.tile([128, 8], f32, tag="saccp")
                lvl = dp_pool.tile([128, 2 * 2048], bf16, tag="lvlp")
                for i, t in enumerate((t0, t0 + 1)):
                    c_lo, c_hi = t * 128, (t + 1) * 128
                    xt_t = xt_pool.tile([128, KT * 128], fp8, tag="xt")
                    nc.sync.dma_start(xt_t[:], xt[c_lo:c_hi, :])
                    x3 = xt_t[:].rearrange("p (j t i) -> p j t i", i=2, t=128)
                    ps = ps_pool.tile([128, NP], f32, tag="ps")
                    for k in range(kt2):
                        for ci, (c0, c1) in enumerate(CHUNKS):
                            nc.tensor.matmul(
                                ps[:, c0:c1], x3[:, k],
                                wt3[:, 2 * k:2 * k + 2, c0:c1],
                                start=(k == 0), stop=False, perf_mode=SW)
                    for ci, (c0, c1) in enumerate(CHUNKS):
                        bp = 32 * ci
                        nc.tensor.matmul(
                            ps[:, c0:c1], ones3[bp:bp + 1, 0, :],
                            wtb3[bp:bp + 1, 0, c0:c1],
                            start=False, stop=True, tile_position=(bp, 0))
                    pen = pen_pool.tile([128, 2 * NP], bf16, tag="pen")
                    nc.scalar.activation(
                        pen[:, NP:2 * NP], ps[:, 0:NP],
                        RELU, scale=1.0 / WSCALE,
                        accum_out=sacc[:, 2 * i:2 * i + 1])
                    nc.scalar.activation(
                        pen[:, 0:NP], ps[:, 0:NP],
                        RELU, scale=-1.0 / WSCALE,
                        accum_out=sacc[:, 2 * i + 1:2 * i + 2])

                    # per-tile tree DP (rotate layout), last level written
                    # into this tile's half of the shared lvl pair tile
                    par = pen[:, 0:2 * NP:NP].rearrange(
                        "p (j u) -> p j u", u=2)
                    pen2 = pen.rearrange("p (s c) -> p s c", s=2)
                    for d in range(1, HEIGHT):
                        w = 1 << d
                        n0 = w - 1
                        if d == HEIGHT - 1:
                            nxt = lvl[:, i * 2048:(i + 1) * 2048]
                        else:
                            nxt_t = dp_pool.tile(
                                [128, 2 * w], bf16, tag=f"lvl{d + 1}")
                            nxt = nxt_t[:]
                        out4 = nxt.rearrange(
                            "p (s j u) -> p s j u", s=2, u=2)
                        pen4 = pen2[:, :, n0:n0 + w].rearrange(
                            "p s (j u) -> p s j u", u=2)
                        par4 = par.rearrange("p (x j) u -> p x j u", x=1)
                        par4 = par4.broadcast_to([128, 2, w // 2, 2])
                        nc.vector.tensor_tensor(out4, par4, pen4, op=ADD)
                        par = nxt.rearrange("p (u j) -> p j u", u=2)

                # batched min tail
                lvl3 = lvl[:].rearrange("p (t s c) -> p t s c", t=2, s=2)
                foldp = dp_pool.tile([128, 4 * 512], bf16, tag="foldp")
                fold3d = foldp[:].rearrange("p (t s c) -> p t s c", t=2, s=2)
                nc.vector.tensor_tensor(
                    fold3d[:, :, :, 0:510], lvl3[:, :, :, 0:510],
                    lvl3[:, :, :, 510:1020], op=MIN)
                nc.vector.tensor_tensor(
                    fold3d[:, :, :, 0:4], fold3d[:, :, :, 0:4],
                    lvl3[:, :, :, 1020:1024], op=MIN)
                nc.vector.tensor_tensor(
                    fold3d[:, :, :, 0:255], fold3d[:, :, :, 0:255],
                    fold3d[:, :, :, 255:510], op=MIN)
                nc.vector.tensor_tensor(
                    fold3d[:, :, :, 0:125], fold3d[:, :, :, 0:125],
                    fold3d[:, :, :, 130:255], op=MIN)
                nc.vector.tensor_tensor(
                    fold3d[:, :, :, 0:5], fold3d[:, :, :, 0:5],
                    fold3d[:, :, :, 125:130], op=MIN)
                tmpp = sm_pool.tile([128, 2 * 10], bf16, tag="minsp")
                tmp4 = tmpp[:].rearrange("p (t s r) -> p t s r", t=2, s=2)
                src = fold3d[:, :, :, 0:130].rearrange(
                    "p t s (j r) -> p t s r j", r=5)
                nc.vector.tensor_reduce(
                    tmp4, src, axis=AXX, op=MIN, negate=True)
                for i, t in enumerate((t0, t0 + 1)):
                    c_lo, c_hi = t * 128, (t + 1) * 128
                    outsb = sm_pool.tile([128, OUT], f32, tag="outsb")
                    out_perm = outsb[:].rearrange("p (r s) -> p s r", s=2)
                    tmp3 = tmpp[:, i * 10:(i + 1) * 10].rearrange(
                        "p (s r) -> p s r", s=2)
                    nc.vector.tensor_scalar(
                        out_perm, tmp3, sacc[:, 2 * i:2 * i + 1],
                        sacc[:, 2 * i + 1:2 * i + 2], op0=ADD, op1=ADD)
                    nc.gpsimd.dma_start(out[c_lo:c_hi, :], outsb[:])

        use_body = body_pair if (pair_dp and mode == "full") else body
        if loop_reps is not None:
            with tc.For_i(0, loop_reps):
                use_body()
        else:
            for _ in range(reps):
                use_body()

    if elide_ldw:
        elide_redundant_ldweights(nc)
    nc.finalize()
    return nc


_NC_CACHE = {}


def _get_nc():
    key = (BT, 1)
    if key not in _NC_CACHE:
        _NC_CACHE[key] = build_kernel()
    return _NC_CACHE[key]


def marshal_xt(in_x_shard):
    """[BC, D] f32 rows -> fp8 [BT*128, KT*128] blocked for the
    DoubleRowSwInterleave stationary layout: SBUF partition p of batch
    tile t holds, per k-step j, 128 interleaved pairs in reversed batch
    order: out[t*128+p, j*256 + 2*q + i] = in_x_shard[t*128 + (127-q),
    j*256 + i*128 + p]."""
    bt = in_x_shard.shape[0] // 128
    a = in_x_shard.reshape(bt, 128, KT2, 2, 128)    # [t, m, j, i, p]
    a = a[:, ::-1]                                  # m -> q = 127-m
    a = a.transpose(0, 4, 2, 1, 3).astype(F8)       # [t, p, j, q, i]
    return np.ascontiguousarray(a.reshape(bt * 128, KT * 128))


def _weight_maps(W1, b1):
    # wt[p, j*NP + n] = W1[n, j*128 + p] * 2^7 (node 2047 column = 0)
    wtf = np.zeros((D, NP), np.float32)
    wtf[:, :N] = W1.T * WSCALE
    wq = wtf.reshape(KT, 128, NP).transpose(1, 0, 2).astype(F8)  # [p, j, n]
    wq = np.ascontiguousarray(wq.reshape(128, KT * NP))
    onesq = np.zeros((4, 2 * 128), np.float32)
    onesq[:, 0:128] = 1.0
    wtbq = np.zeros((4, 2 * NP), np.float32)
    wtbq[:, :N] = b1 * WSCALE
    return wq, onesq.astype(F8), wtbq.astype(F8)


def make_in_map(in_x_shard, W1, b1):
    wq, onesq, wtbq = _weight_maps(W1, b1)
    return {"xt": marshal_xt(np.asarray(in_x_shard, np.float32)),
            "wt": wq, "ones": onesq, "wtb": wtbq}


def kernel(in_x, W1, b1, L, A):
    in_x = np.asarray(in_x, np.float32)
    W1 = np.asarray(W1, np.float32)
    b1 = np.asarray(b1, np.float32)
    wq, onesq, wtbq = _weight_maps(W1, b1)
    in_maps = [
        {"xt": marshal_xt(in_x[c * BC:(c + 1) * BC]), "wt": wq,
         "ones": onesq, "wtb": wtbq}
        for c in range(NCORES)
    ]
    nc = _get_nc()
    res = run_bass_kernel_spmd(nc, in_maps, core_ids=list(range(NCORES)))
    return np.concatenate([res.results[c]["out"] for c in range(NCORES)], axis=0)



# revision 39
# speedup vs baseline: 1.0117x; 1.0117x over previous
"""DTSemNet forward (nn_DTSemNet_54528904790526) on 8 TRN2 NeuronCores.

Math: the reference computes
    x = in_x @ W1.T + b1                       [B, 2047]
    h = [relu(x), relu(-x)]                    [B, 4094]
    z = h @ L.T                                [B, 2048]   (frozen 0/1 leaf routing)
    out[b, a] = max over leaves ell with (ell % 10 == a) of z[b, ell]

L is the complete-binary-tree path matrix, so
    z[b, ell] = sum_i |x_i|  -  sum_{path nodes} penalty,
    penalty   = relu(-x_node) going left, relu(x_node) going right,
replacing the dense [B,4094]x[4094,2048] matmul with an 11-level tree DP
on the vector engine (see kernel_fp32r_baseline.py for the derivation).

This version runs the linear1 matmul in fp8 (e4m3) with
MatmulPerfMode.DoubleRowSwInterleave: 2 fp8 weights per PE cell ->
256-deep contraction per instruction; the software-interleaved
(pair-interleaved, column-reversed) stationary layout loads weights
contiguously. Inputs are quantized host-side: activations cast straight
to e4m3 (|x|<6 << 240, TRN e4m3 infs at 256), weights and bias
pre-scaled by 2^7 (exact) to clear the e4m3 subnormal floor at 2^-6;
the relu undoes the scale exactly via the activation scale (+-2^-7).
Measured end-to-end error: max rel 8.1e-3 vs the 2e-2 gate (matmul
accumulates in fp32 PSUM; e4m3 products are exact in fp32).

Per-core shard: batch rows (data parallel over 8 cores, 2048 rows each).
Per 128-row batch tile: 8 SwInterleave k-steps x 4 chunk matmuls
(k-outer so 4 consecutive matmuls share the stationary) accumulate
x*2^7 into one [128,2048] PSUM tile (4 banks); the bias row is folded
in as a K=1 ones-row NORMAL-mode (FWL) matmul per chunk at 4 PE
row-group positions -- HW-measured ~13 us/sweep faster than the
DoubleRow bias matmuls of the earlier version. Two full-width
activations produce pen = relu(+-x) bf16 with accumulated row-sums
(S_abs). The 11-level tree DP runs on DVE in bf16 at ~0.86 elem/cyc
(the rotate layout that keeps tree-order m within halves forces a
non-contiguous innermost par operand, which blocks DVE 2X packing;
every contiguous alternative bit-reverses the leaf order and breaks
the mod-5 group reduce -- verified impossible with affine APs). The
group-min is a 3-stage fold chain 1020->510->255->130 (fold offsets
510, 255, 130 are all = 0 mod 5, preserving residues) + leftover
fixups + one 4-D strided reduce [s][r=5][j=26] + the final
(-min+sacc0)+sacc1 combine, emitted inline (tail_depth=0: with the
folds this small, deferring the tail one tile measured slightly worse
-- longer dp-tile lifetimes -- where it had been a win when the reduce
read 1020 elements). Folds 2+3 measured ~8-11 us/sweep combined.

Corrected HW model (this session, measured by deltas + NEFF disasm +
CoreSim): LDWEIGHTS is NOT the matmul floor -- eliding 24/32 redundant
LDWs per tile (elide_redundant_ldweights below, kept on) changed
nothing; the stream is bound by the DoubleRow MATMUL ALU time itself,
~175 ns marginal per 512-col MM (k-slope probes). Chunk-outer MM order
(per-MM stationary reload) costs +26 ns/MM => k-outer stays. Pool (Q7)
is useless for compute here: TT add 0.5 elem/cyc, no TT-min/max, no
PSUM reads, TS ~0.07 elem/cyc -- it only carries the output DMAs.
Offloading DP levels/halves to Pool measured strictly worse.

Measured on this container (For_i repeat-loop deltas, fresh process
per measurement; device has +-5% state noise): ~141-148 us/core-sweep
vs ~150 us for the previous version and 248.6 us for the fp32r
baseline. mmpure floor ~112 us (main MMs ~91 us + fixed ~21 us of
DMA/drain/gaps); the For_i all-engine barrier per iteration makes each
sweep pay the full ACT+DVE drain (~12 us), which a single harness
invocation pays once too. Other dead ends measured this session:
deeper pen/dp/sm/xt rings (no gain; DVE-saturated per CoreSim -- ACT
idle is not binding), xt DMA on an alternate (ACT) queue (+19 us),
half-width ACT splits (+3 us), pre-materializing the DP par
duplication via SBUF->SBUF strided DMA to unlock DVE 2X packing
(the 2-byte-stride DMA runs at ~100 ns/element -- useless), and
pair-batching the DVE ops across two tiles to amortize the ~110ns
per-op overhead (level TTs blocked: walrus DVE codegen is TENSOR3D,
max 3 free dims, and the broadcast par dim can't merge with a tile
dim; tail-only batching built and correct but measured ~2-4 us WORSE
-- the shared lvl pair tile serializes the two tiles' DP chains).
"""
import sys

sys.path.insert(0, "/opt/trn_rl_repo")
from contextlib import ExitStack

import numpy as np
import ml_dtypes

import concourse.bass as bass
import concourse.tile as tile
from concourse import bacc, mybir
from concourse.bass_utils import run_bass_kernel_spmd

# problem shape (hardcoded per contract)
B = 16384
D = 2048
N = 2047          # internal nodes
NP = 2048         # N padded (zero column 2047)
HEIGHT = 11
NL = 2048         # leaves
OUT = 10
NCORES = 8
BC = B // NCORES  # batch rows per core (2048)
KT = D // 128     # 16 k-subtiles of 128 contraction rows
KT2 = KT // 2     # 8 DoubleRow k-steps of 256
BT = BC // 128    # 16 batch tiles per core
CHUNKS = [(0, 512), (512, 1024), (1024, 1536), (1536, 2048)]
WSCALE = 128.0    # weight pre-scale 2^7 (exact in fp8/fp32)

F8 = ml_dtypes.float8_e4m3   # TRN fp8e4: inf at S.1111.000, max normal 240

f32 = mybir.dt.float32
fp8 = mybir.dt.float8e4
bf16 = mybir.dt.bfloat16
ADD = mybir.AluOpType.add
MIN = mybir.AluOpType.min
SUB = mybir.AluOpType.subtract
MULT = mybir.AluOpType.mult
RELU = mybir.ActivationFunctionType.Relu
AXX = mybir.AxisListType.X
DR = mybir.MatmulPerfMode.DoubleRow
SW = mybir.MatmulPerfMode.DoubleRowSwInterleave


def elide_redundant_ldweights(nc):
    """Drop InstLdweights that reload the stationary already in the PE.

    The Tile scheduler emits one Ldweights+Matmult pair per matmul; the 4
    chunk matmuls of one k-step share the same stationary (x3[:, k]), so
    3 of every 4 Ldweights reload identical weights (~107ns each on the
    serial PE queue). Walrus lowers InstMatmult to a bare MATMUL that uses
    whatever stationary is loaded, so removing a redundant Ldweights is
    safe when (a) its weights AP + tile geometry match the previously
    loaded one with only Matmults in between on the PE queue, and (b) it
    carries no semaphore waits/updates.
    """
    PE = mybir.EngineType.PE
    removed = 0
    for blk in nc.main_func.blocks:
        il = blk.instructions
        cur_sig = None
        drop = set()
        for i in il:
            if getattr(i, "engine", None) != PE:
                continue
            t = type(i).__name__
            if t == "InstLdweights":
                sig = (str(i.ins[0]), str(i.tile_position),
                       str(i.tile_size), str(i.perf_mode),
                       str(i.is_transpose))
                si = i.sync_info
                clean = si is None or (len(si.on_wait) == 0
                                       and len(si.on_update) == 0)
                if clean and sig == cur_sig:
                    drop.add(id(i))
                    removed += 1
                else:
                    cur_sig = sig
            elif t == "InstMatmult":
                pass          # uses loaded stationary, doesn't change it
            else:
                cur_sig = None  # unknown PE instruction: invalidate
        if drop:
            blk.instructions = [i for i in il if id(i) not in drop]
    return removed


def build_kernel(bt=BT, reps=1, loop_reps=None, mode="full", elide_ldw=True,
                 kt2=KT2, korder="k", biasfirst=False,
                 pool_split=False, defer_fold=False, fold2x=True,
                 bias_normal=True, tail_depth=0, pool_dmax=0, fold3x=True,
                 act_split=False, xt_bufs=6, pen_bufs=2, dp_bufs=3,
                 sm_bufs=4, xt_alt_queue=False, pair_dp=False):
    """bt: number of batch tiles (128 rows each) this kernel processes.
    reps: python-unrolled repeats of the whole per-tile pipeline.
    loop_reps: device-side For_i repeats (for timing probes).
    mode: "full" | "nodp" (skip tree DP/mins) | "mmonly" (matmuls only)
          | "dponly" (memset pen, DP chain only)."""
    nc = bacc.Bacc("TRN2")
    # in_x fp8 shard, pre-blocked host-side as [bt][128 p][KT j][128 m]
    # with p = contraction row % 128, j = contraction row // 128, so each
    # SBUF partition reads one contiguous 2KB run per batch tile.
    xt = nc.dram_tensor("xt", [bt * 128, KT * 128], fp8, kind="ExternalInput")
    # W1.T * 2^7 (zero col at node 2047), blocked [128 p][KT j][NP n]
    wt = nc.dram_tensor("wt", [128, KT * NP], fp8, kind="ExternalInput")
    # ones rows for the bias matmul: [1.0]*128 | [0.0]*128 per row
    ones = nc.dram_tensor("ones", [4, 2 * 128], fp8, kind="ExternalInput")
    # bias rows: b1 * 2^7 (2048, zero-padded) | zeros(2048) per row
    wtb = nc.dram_tensor("wtb", [4, 2 * NP], fp8, kind="ExternalInput")
    out = nc.dram_tensor("out", [bt * 128, OUT], f32, kind="ExternalOutput")

    with tile.TileContext(nc) as tc, ExitStack() as ctx:
        wt_pool = ctx.enter_context(tc.tile_pool(name="wt", bufs=1))
        xt_pool = ctx.enter_context(tc.tile_pool(name="xt", bufs=xt_bufs))
        ps_pool = ctx.enter_context(tc.tile_pool(name="ps", bufs=2, space="PSUM"))
        pen_pool = ctx.enter_context(tc.tile_pool(name="pen", bufs=pen_bufs))
        dp_pool = ctx.enter_context(tc.tile_pool(name="dp", bufs=dp_bufs))
        sm_pool = ctx.enter_context(tc.tile_pool(name="sm", bufs=sm_bufs))

        # resident weights: [128, KT, NP] fp8 (32KB/partition)
        wts = wt_pool.tile([128, KT * NP], fp8, tag="wt")
        nc.sync.dma_start(wts[:], wt[:, :])
        wt3 = wts[:].rearrange("p (j n) -> p j n", n=NP)
        # bias + ones rows replicated at partitions 0/32/64/96 so the four
        # K=1 bias matmuls can run as concurrent PE row-group tiles
        ones_t = wt_pool.tile([128, 2 * 128], fp8, tag="ones")
        wtb_t = wt_pool.tile([128, 2 * NP], fp8, tag="wtb")
        for rg in range(4):
            nc.sync.dma_start(ones_t[32 * rg:32 * rg + 1, :], ones[rg:rg + 1, :])
            nc.sync.dma_start(wtb_t[32 * rg:32 * rg + 1, :], wtb[rg:rg + 1, :])
        ones3 = ones_t[:].rearrange("p (i m) -> p i m", m=128)
        wtb3 = wtb_t[:].rearrange("p (i n) -> p i n", n=NP)

        def body():
            pending = []
            for t in range(bt):
                c_lo = t * 128
                c_hi = (t + 1) * 128
                xt_t = xt_pool.tile([128, KT * 128], fp8, tag="xt")
                if xt_alt_queue and t % 2 == 1:
                    nc.scalar.dma_start(xt_t[:], xt[c_lo:c_hi, :])
                else:
                    nc.sync.dma_start(xt_t[:], xt[c_lo:c_hi, :])
                # SwInterleave stationary layout: per k-step, pairs
                # (sub0, sub1) interleaved per batch column, columns reversed
                x3 = xt_t[:].rearrange("p (j t i) -> p j t i", i=2, t=128)

                pen = pen_pool.tile([128, 2 * NP], bf16, tag="pen")
                sacc = sm_pool.tile([128, 4], f32, tag="sacc")

                if mode == "dponly":
                    # fill pen/sacc cheaply so the DP chain is isolated
                    nc.vector.memset(pen[:], 0.5)
                    nc.vector.memset(sacc[:], 1.0)
                else:
                    ps = ps_pool.tile([128, NP], f32, tag="ps")
                    if mode == "mmnodma":
                        nc.vector.memset(xt_t[:], 0.25)
                    skip_bias = mode in ("mmnob", "mmnobias")
                    if biasfirst and not skip_bias:
                        for ci, (c0, c1) in enumerate(CHUNKS):
                            bp = 32 * ci
                            nc.tensor.matmul(
                                ps[:, c0:c1],
                                ones3[bp:bp + 1, :, :],
                                wtb3[bp:bp + 1, :, c0:c1],
                                start=True, stop=False,
                                perf_mode=DR,
                                tile_position=(bp, 0),
                            )
                    if korder == "chunk":
                        # chunk-outer: each chunk's 8 k-steps consecutive
                        for ci, (c0, c1) in enumerate(CHUNKS):
                            for k in range(kt2):
                                nc.tensor.matmul(
                                    ps[:, c0:c1],
                                    x3[:, k],
                                    wt3[:, 2 * k:2 * k + 2, c0:c1],
                                    start=(k == 0 and not biasfirst),
                                    stop=(k == kt2 - 1
                                          and (biasfirst or skip_bias)),
                                    perf_mode=SW,
                                )
                    else:
                        # k-outer order: 4 consecutive matmuls share the
                        # stationary xt block
                        for k in range(kt2):
                            for ci, (c0, c1) in enumerate(CHUNKS):
                                nc.tensor.matmul(
                                    ps[:, c0:c1],
                                    x3[:, k],
                                    wt3[:, 2 * k:2 * k + 2, c0:c1],
                                    start=(k == 0 and not biasfirst),
                                    stop=(k == kt2 - 1
                                          and (biasfirst or skip_bias)),
                                    perf_mode=SW,
                                )
                    if not biasfirst and not skip_bias:
                        for ci, (c0, c1) in enumerate(CHUNKS):
                            bp = 32 * ci
                            if bias_normal:
                                nc.tensor.matmul(
                                    ps[:, c0:c1],
                                    ones3[bp:bp + 1, 0, :],
                                    wtb3[bp:bp + 1, 0, c0:c1],
                                    start=False, stop=True,
                                    tile_position=(bp, 0),
                                )
                            else:
                                nc.tensor.matmul(
                                    ps[:, c0:c1],
                                    ones3[bp:bp + 1, :, :],
                                    wtb3[bp:bp + 1, :, c0:c1],
                                    start=False, stop=True,
                                    perf_mode=DR,
                                    tile_position=(bp, 0),
                                )
                    if mode in ("mmpure", "mmnobias"):
                        continue
                    if mode in ("mmonly", "mmnob", "mmnodma"):
                        outsb = sm_pool.tile([128, OUT], f32, tag="outsb")
                        nc.scalar.copy(outsb[:], ps[:, 0:OUT])
                        nc.sync.dma_start(out[c_lo:c_hi, :], outsb[:])
                        continue
                    # pen = [relu(-x) | relu(x)] with running row-sums;
                    # the 2^-7 scale undoes the weight pre-scale exactly.
                    # act_split="last": half-width ACT pairs for the final
                    # tile only -- the DP levels 1..9 need only the lo node
                    # half, so the last tile's drain chain starts ~2us
                    # earlier while the other 15 tiles keep the cheaper
                    # full-width ACTs.
                    if act_split is True or (act_split == "last"
                                             and t == bt - 1):
                        # lo halves first (nodes 0:1024) so the DP levels
                        # 1..9 can start while the hi halves run
                        H2 = NP // 2
                        nc.scalar.activation(
                            pen[:, NP:NP + H2], ps[:, 0:H2], RELU,
                            scale=1.0 / WSCALE, accum_out=sacc[:, 0:1])
                        nc.scalar.activation(
                            pen[:, 0:H2], ps[:, 0:H2], RELU,
                            scale=-1.0 / WSCALE, accum_out=sacc[:, 1:2])
                        nc.scalar.activation(
                            pen[:, NP + H2:2 * NP], ps[:, H2:NP], RELU,
                            scale=1.0 / WSCALE, accum_out=sacc[:, 2:3])
                        nc.scalar.activation(
                            pen[:, H2:NP], ps[:, H2:NP], RELU,
                            scale=-1.0 / WSCALE, accum_out=sacc[:, 3:4])
                        # sacc[0:2] += sacc[2:4] so the final TS still uses
                        # just two scalars
                        nc.vector.tensor_tensor(
                            sacc[:, 0:2], sacc[:, 0:2], sacc[:, 2:4], op=ADD)
                    else:
                        nc.scalar.activation(
                            pen[:, NP:2 * NP], ps[:, 0:NP], RELU,
                            scale=1.0 / WSCALE,
                            accum_out=sacc[:, 0:1],
                        )
                        nc.scalar.activation(
                            pen[:, 0:NP], ps[:, 0:NP], RELU,
                            scale=-1.0 / WSCALE,
                            accum_out=sacc[:, 1:2],
                        )

                if mode in ("nodp", "nomin"):
                    sabs = sm_pool.tile([128, 1], f32, tag="sabs")
                    nc.vector.tensor_tensor(
                        sabs[:, 0:1], sacc[:, 0:1], sacc[:, 1:2], op=ADD)
                if mode == "nodp":
                    outsb = sm_pool.tile([128, OUT], f32, tag="outsb")
                    nc.scalar.copy(outsb[:, 0:1], sabs[:])
                    nc.sync.dma_start(out[c_lo:c_hi, 0:1], outsb[:, 0:1])
                    continue

                # ---- tree DP over 11 levels, split (evens|odds) layout ----
                # One TT per level: out[s, j, u] = par[j, u] + pen[s][n0 + 2j+u]
                # (s = 0 left / 1 right half; parent broadcast via stride-0 dim)
                # level-1 costs are pen[0] (left child) and pen[NP] (right)
                par = pen[:, 0:2 * NP:NP].rearrange("p (j u) -> p j u", u=2)
                pen2 = pen.rearrange("p (s c) -> p s c", s=2)
                lvl = None
                for d in range(1, HEIGHT):
                    w = 1 << d          # number of level-d nodes = parents
                    n0 = w - 1          # first node index of level d
                    nxt = dp_pool.tile([128, 2 * w], bf16, tag=f"lvl{d + 1}")
                    if d == HEIGHT - 1 and pool_split:
                        # final (biggest) level: split halves across engines
                        # -- DVE takes s=0, the otherwise-idle Pool s=1 --
                        # to shed ~1.2us/tile of DVE time. par is shared.
                        for s, eng in ((0, nc.vector), (1, nc.gpsimd)):
                            out3 = nxt[:, s * w:(s + 1) * w].rearrange(
                                "p (j u) -> p j u", u=2)
                            pen3 = pen[:, s * NP + n0:s * NP + n0 + w
                                       ].rearrange("p (j u) -> p j u", u=2)
                            eng.tensor_tensor(out3, par, pen3, op=ADD)
                    else:
                        out4 = nxt[:].rearrange(
                            "p (s j u) -> p s j u", s=2, u=2)
                        pen4 = pen2[:, :, n0:n0 + w].rearrange(
                            "p s (j u) -> p s j u", u=2)
                        par4 = par.rearrange("p (x j) u -> p x j u", x=1)
                        par4 = par4.broadcast_to([128, 2, w // 2, 2])
                        eng = nc.gpsimd if d <= pool_dmax else nc.vector
                        eng.tensor_tensor(out4, par4, pen4, op=ADD)
                    lvl = nxt
                    par = nxt[:].rearrange("p (u j) -> p j u", u=2)

                if mode == "nomin":
                    outsb = sm_pool.tile([128, OUT], f32, tag="outsb")
                    nc.vector.tensor_scalar(
                        outsb[:], lvl[:, 0:OUT], sabs[:, 0:1], -1.0,
                        op0=SUB, op1=MULT,
                    )
                    nc.sync.dma_start(out[c_lo:c_hi, :], outsb[:])
                    continue

                # lvl holds leaf costs [128, 2048]: evens | odds halves.
                # group a=2r+s: min over positions m ≡ r (mod 5) of half s.
                # The ENTIRE min tail (folds + reduce + combine + store) is
                # deferred one tile (software pipelining): the Pool half of
                # the last DP level lands late, and an in-line fold would
                # head-of-line-block the DVE FIFO waiting on it.
                # Fold chain: 1020 -> 510 -> 255 (both 510 and 255 are
                # ≡ 0 mod 5, so residues are preserved), leftover
                # m=1020..1023 (residues 0..3) folded into the matching
                # residue slots, then one 4-D strided reduce
                # [s][r=5][j=51] -> tmp[s*5+r] with negate, and the final
                # (-min + sacc0) + sacc1 combine.
                def do_folds(lvl2, fold2):
                    nc.vector.tensor_tensor(
                        fold2[:, :, 0:510], lvl2[:, :, 0:510],
                        lvl2[:, :, 510:1020], op=MIN,
                    )
                    nc.vector.tensor_tensor(
                        fold2[:, :, 0:4], fold2[:, :, 0:4],
                        lvl2[:, :, 1020:1024], op=MIN,
                    )
                    if fold2x:
                        nc.vector.tensor_tensor(
                            fold2[:, :, 0:255], fold2[:, :, 0:255],
                            fold2[:, :, 255:510], op=MIN,
                        )
                    if fold3x:
                        # 255 -> 130: pairs (i, i+130), 130 ≡ 0 mod 5;
                        # leftover 125:130 (residues 0..4) folds into 0:5
                        nc.vector.tensor_tensor(
                            fold2[:, :, 0:125], fold2[:, :, 0:125],
                            fold2[:, :, 130:255], op=MIN,
                        )
                        nc.vector.tensor_tensor(
                            fold2[:, :, 0:5], fold2[:, :, 0:5],
                            fold2[:, :, 125:130], op=MIN,
                        )

                pre_fold2 = None
                if not defer_fold:
                    lvl2_i = lvl[:].rearrange("p (s c) -> p s c", s=2)
                    fold_i = dp_pool.tile([128, 2 * 512], bf16, tag="fold")
                    pre_fold2 = fold_i[:].rearrange("p (s c) -> p s c", s=2)
                    do_folds(lvl2_i, pre_fold2)

                def min_tail(lvl=lvl, sacc=sacc, c_lo=c_lo, c_hi=c_hi,
                             pre_fold2=pre_fold2):
                    if pre_fold2 is None:
                        lvl2 = lvl[:].rearrange("p (s c) -> p s c", s=2)
                        fold = dp_pool.tile([128, 2 * 512], bf16, tag="fold")
                        fold2 = fold[:].rearrange("p (s c) -> p s c", s=2)
                        do_folds(lvl2, fold2)
                    else:
                        fold2 = pre_fold2
                    nred = 130 if fold3x else (255 if fold2x else 510)
                    tmp = sm_pool.tile([128, 2 * 5], bf16, tag="mins")
                    tmp3 = tmp[:].rearrange("p (s r) -> p s r", s=2)
                    src = fold2[:, :, 0:nred].rearrange(
                        "p s (j r) -> p s r j", r=5)
                    nc.vector.tensor_reduce(
                        tmp3, src, axis=AXX, op=MIN, negate=True)
                    outsb = sm_pool.tile([128, OUT], f32, tag="outsb")
                    # out[:, 2r+s] = (-min[s*5+r] + sacc0) + sacc1
                    out_perm = outsb[:].rearrange("p (r s) -> p s r", s=2)
                    nc.vector.tensor_scalar(
                        out_perm, tmp3, sacc[:, 0:1], sacc[:, 1:2],
                        op0=ADD, op1=ADD,
                    )
                    # out DMA via the idle Pool SWDGE: it depends on the late
                    # DVE tail, and on the SP queue it would head-of-line
                    # block the next tiles' xt input loads
                    nc.gpsimd.dma_start(out[c_lo:c_hi, :], outsb[:])

                pending.append(min_tail)
                if len(pending) > tail_depth:
                    pending.pop(0)()
            for fn in pending:
                fn()

        def body_pair():
            # Pair-batched tail: walrus DVE codegen is TENSOR3D (max 3 free
            # dims), and the level TTs' broadcast par dim cannot merge with
            # a tile dim, so the DP levels stay per-tile. But the fold and
            # reduce ops have no broadcast operands -- their (t, s) dims
            # merge -- so one tail per tile PAIR halves tail op count and
            # semaphore traffic (~6 DVE ops saved per pair).
            for t0 in range(0, bt, 2):
                sacc = sm_pool.tile([128, 8], f32, tag="saccp")
                lvl = dp_pool.tile([128, 2 * 2048], bf16, tag="lvlp")
                for i, t in enumerate((t0, t0 + 1)):
                    c_lo, c_hi = t * 128, (t + 1) * 128
                    xt_t = xt_pool.tile([128, KT * 128], fp8, tag="xt")
                    nc.sync.dma_start(xt_t[:], xt[c_lo:c_hi, :])
                    x3 = xt_t[:].rearrange("p (j t i) -> p j t i", i=2, t=128)
                    ps = ps_pool.tile([128, NP], f32, tag="ps")
                    for k in range(kt2):
                        for ci, (c0, c1) in enumerate(CHUNKS):
                            nc.tensor.matmul(
                                ps[:, c0:c1], x3[:, k],
                                wt3[:, 2 * k:2 * k + 2, c0:c1],
                                start=(k == 0), stop=False, perf_mode=SW)
                    for ci, (c0, c1) in enumerate(CHUNKS):
                        bp = 32 * ci
                        nc.tensor.matmul(
                            ps[:, c0:c1], ones3[bp:bp + 1, 0, :],
                            wtb3[bp:bp + 1, 0, c0:c1],
                            start=False, stop=True, tile_position=(bp, 0))
                    pen = pen_pool.tile([128, 2 * NP], bf16, tag="pen")
                    nc.scalar.activation(
                        pen[:, NP:2 * NP], ps[:, 0:NP],
                        RELU, scale=1.0 / WSCALE,
                        accum_out=sacc[:, 2 * i:2 * i + 1])
                    nc.scalar.activation(
                        pen[:, 0:NP], ps[:, 0:NP],
                        RELU, scale=-1.0 / WSCALE,
                        accum_out=sacc[:, 2 * i + 1:2 * i + 2])

                    # per-tile tree DP (rotate layout), last level written
                    # into this tile's half of the shared lvl pair tile
                    par = pen[:, 0:2 * NP:NP].rearrange(
                        "p (j u) -> p j u", u=2)
                    pen2 = pen.rearrange("p (s c) -> p s c", s=2)
                    for d in range(1, HEIGHT):
                        w = 1 << d
                        n0 = w - 1
                        if d == HEIGHT - 1:
                            nxt = lvl[:, i * 2048:(i + 1) * 2048]
                        else:
                            nxt_t = dp_pool.tile(
                                [128, 2 * w], bf16, tag=f"lvl{d + 1}")
                            nxt = nxt_t[:]
                        out4 = nxt.rearrange(
                            "p (s j u) -> p s j u", s=2, u=2)
                        pen4 = pen2[:, :, n0:n0 + w].rearrange(
                            "p s (j u) -> p s j u", u=2)
                        par4 = par.rearrange("p (x j) u -> p x j u", x=1)
                        par4 = par4.broadcast_to([128, 2, w // 2, 2])
                        nc.vector.tensor_tensor(out4, par4, pen4, op=ADD)
                        par = nxt.rearrange("p (u j) -> p j u", u=2)

                # batched min tail
                lvl3 = lvl[:].rearrange("p (t s c) -> p t s c", t=2, s=2)
                foldp = dp_pool.tile([128, 4 * 512], bf16, tag="foldp")
                fold3d = foldp[:].rearrange("p (t s c) -> p t s c", t=2, s=2)
                nc.vector.tensor_tensor(
                    fold3d[:, :, :, 0:510], lvl3[:, :, :, 0:510],
                    lvl3[:, :, :, 510:1020], op=MIN)
                nc.vector.tensor_tensor(
                    fold3d[:, :, :, 0:4], fold3d[:, :, :, 0:4],
                    lvl3[:, :, :, 1020:1024], op=MIN)
                nc.vector.tensor_tensor(
                    fold3d[:, :, :, 0:255], fold3d[:, :, :, 0:255],
                    fold3d[:, :, :, 255:510], op=MIN)
                nc.vector.tensor_tensor(
                    fold3d[:, :, :, 0:125], fold3d[:, :, :, 0:125],
                    fold3d[:, :, :, 130:255], op=MIN)
                nc.vector.tensor_tensor(
                    fold3d[:, :, :, 0:5], fold3d[:, :, :, 0:5],
                    fold3d[:, :, :, 125:130], op=MIN)
                tmpp = sm_pool.tile([128, 2 * 10], bf16, tag="minsp")
                tmp4 = tmpp[:].rearrange("p (t s r) -> p t s r", t=2, s=2)
                src = fold3d[:, :, :, 0:130].rearrange(
                    "p t s (j r) -> p t s r j", r=5)
                nc.vector.tensor_reduce(
                    tmp4, src, axis=AXX, op=MIN, negate=True)
                for i, t in enumerate((t0, t0 + 1)):
                    c_lo, c_hi = t * 128, (t + 1) * 128
                    outsb = sm_pool.tile([128, OUT], f32, tag="outsb")
                    out_perm = outsb[:].rearrange("p (r s) -> p s r", s=2)
                    tmp3 = tmpp[:, i * 10:(i + 1) * 10].rearrange(
                        "p (s r) -> p s r", s=2)
                    nc.vector.tensor_scalar(
                        out_perm, tmp3, sacc[:, 2 * i:2 * i + 1],
                        sacc[:, 2 * i + 1:2 * i + 2], op0=ADD, op1=ADD)
                    nc.gpsimd.dma_start(out[c_lo:c_hi, :], outsb[:])

        use_body = body_pair if (pair_dp and mode == "full") else body
        if loop_reps is not None:
            with tc.For_i(0, loop_reps):
                use_body()
        else:
            for _ in range(reps):
                use_body()

    if elide_ldw:
        elide_redundant_ldweights(nc)
    nc.finalize()
    return nc


_NC_CACHE = {}


def _get_nc():
    key = (BT, 1)
    if key not in _NC_CACHE:
        _NC_CACHE[key] = build_kernel()
    return _NC_CACHE[key]


def marshal_xt(in_x_shard):
    """[BC, D] f32 rows -> fp8 [BT*128, KT*128] blocked for the
    DoubleRowSwInterleave stationary layout: SBUF partition p of batch
    tile t holds, per k-step j, 128 interleaved pairs in reversed batch
    order: out[t*128+p, j*256 + 2*q + i] = in_x_shard[t*128 + (127-q),
    j*256 + i*128 + p]."""
    bt = in_x_shard.shape[0] // 128
    a = in_x_shard.reshape(bt, 128, KT2, 2, 128)    # [t, m, j, i, p]
    a = a[:, ::-1]                                  # m -> q = 127-m
    a = a.transpose(0, 4, 2, 1, 3).astype(F8)       # [t, p, j, q, i]
    return np.ascontiguousarray(a.reshape(bt * 128, KT * 128))


def _weight_maps(W1, b1):
    # wt[p, j*NP + n] = W1[n, j*128 + p] * 2^7 (node 2047 column = 0)
    wtf = np.zeros((D, NP), np.float32)
    wtf[:, :N] = W1.T * WSCALE
    wq = wtf.reshape(KT, 128, NP).transpose(1, 0, 2).astype(F8)  # [p, j, n]
    wq = np.ascontiguousarray(wq.reshape(128, KT * NP))
    onesq = np.zeros((4, 2 * 128), np.float32)
    onesq[:, 0:128] = 1.0
    wtbq = np.zeros((4, 2 * NP), np.float32)
    wtbq[:, :N] = b1 * WSCALE
    return wq, onesq.astype(F8), wtbq.astype(F8)


def make_in_map(in_x_shard, W1, b1):
    wq, onesq, wtbq = _weight_maps(W1, b1)
    return {"xt": marshal_xt(np.asarray(in_x_shard, np.float32)),
            "wt": wq, "ones": onesq, "wtb": wtbq}


def kernel(in_x, W1, b1, L, A):
    in_x = np.asarray(in_x, np.float32)
    W1 = np.asarray(W1, np.float32)
    b1 = np.asarray(b1, np.float32)
    wq, onesq, wtbq = _weight_maps(W1, b1)
    in_maps = [
        {"xt": marshal_xt(in_x[c * BC:(c + 1) * BC]), "wt": wq,
         "ones": onesq, "wtb": wtbq}
        for c in range(NCORES)
    ]
    nc = _get_nc()
    res = run_bass_kernel_spmd(nc, in_maps, core_ids=list(range(NCORES)))
    return np.concatenate([res.results[c]["out"] for c in range(NCORES)], axis=0)

